# revision 6
# baseline (speedup 1.0000x reference)
"""GAT layer (gnn_message_passing) on 8 Trainium2 NeuronCores.

Sharding: nodes are partitioned contiguously across the 8 cores; edges are
assigned to the core that owns their dst node so the edge-softmax segments
and the weighted scatter-sum stay core-local.  Each core builds the full
k/v table on-device (dense weights replicated, no collectives), gathers
k/v[src] + q[dst] rows per edge with indirect DMA, and does all segment
operations as indicator-matrix matmuls accumulating in PSUM.
"""

import sys

sys.path.insert(0, "/opt/trn_rl_repo")

import math

import numpy as np

import concourse.bass as bass
import concourse.tile as tile
from concourse import bacc, mybir
from concourse.bass import IndirectOffsetOnAxis
from concourse.bass_utils import run_bass_kernel_spmd

NCORES = 8
P = 128          # partitions / tile node count / matmul K
H = 8            # heads
DH = 16          # head dim
FC_P = 128       # ffn chunk size

F32 = mybir.dt.float32
F16 = mybir.dt.float16
I32 = mybir.dt.int32


# ---------------------------------------------------------------- host prep

def _preprocess(feat, src, dst):
    """Sort/shard edges by dst, pad everything to uniform per-tile shapes."""
    N, D = feat.shape
    E = src.shape[0]
    NL = int(math.ceil(N / (NCORES * P))) * P      # nodes per core (padded)
    NP = NL * NCORES                               # padded total nodes
    T = NL // P                                    # node tiles per core

    feat_pad = np.zeros((NP, D), np.float32)
    feat_pad[:N] = feat

    order = np.argsort(dst, kind="stable")
    s_sorted = src[order].astype(np.int64)
    d_sorted = dst[order].astype(np.int64)

    gtile = d_sorted // P                          # global tile id of each edge
    counts = np.bincount(gtile, minlength=NP // P)
    CM = max(1, int(math.ceil(counts.max() / P)))  # chunks per tile (uniform)

    starts = np.concatenate([[0], np.cumsum(counts)])
    pos = np.arange(E) - starts[gtile]
    p_slot = pos % P
    j_slot = pos // P
    core = gtile // T
    col = (gtile % T) * CM + j_slot

    idx_kv = np.zeros((NCORES, P, T * CM), np.int32)
    idx_q = np.zeros((NCORES, P, T * CM), np.int32)
    dst_rel = np.full((NCORES, P, T * CM), -1.0, np.float16)
    idx_kv[core, p_slot, col] = s_sorted
    idx_q[core, p_slot, col] = d_sorted % NL
    dst_rel[core, p_slot, col] = (d_sorted % P).astype(np.float16)

    return dict(N=N, D=D, NL=NL, NP=NP, T=T, CM=CM, feat_pad=feat_pad,
                idx_kv=idx_kv, idx_q=idx_q, dst_rel=dst_rel)


def _rep(v, rows=P):
    """Replicate a 1-D vector across partitions -> [rows, len(v)] f32."""
    v = np.asarray(v, np.float32).reshape(1, -1)
    return np.ascontiguousarray(np.repeat(v, rows, axis=0))


# ---------------------------------------------------------------- program

def build_program(NP, NL, T, CM, D, F, repeat=1, debug=False):
    """One SPMD program; all per-core differences live in the input data."""
    FC = F // FC_P
    nc = bacc.Bacc("TRN2", target_bir_lowering=False, debug=False,
                   num_devices=NCORES)

    feat_full = nc.dram_tensor("feat_full", [NP, D], F32, kind="ExternalInput")
    feat_loc = nc.dram_tensor("feat_loc", [NL, D], F32, kind="ExternalInput")
    idx_kv_d = nc.dram_tensor("idx_kv", [P, T * CM], I32, kind="ExternalInput")
    idx_q_d = nc.dram_tensor("idx_q", [P, T * CM], I32, kind="ExternalInput")
    dst_rel_d = nc.dram_tensor("dst_rel", [P, T * CM], F16, kind="ExternalInput")
    wq_d = nc.dram_tensor("Wq", [D, D], F32, kind="ExternalInput")
    wk_d = nc.dram_tensor("Wk", [D, D], F32, kind="ExternalInput")
    wv_d = nc.dram_tensor("Wv", [D, D], F32, kind="ExternalInput")
    w1_d = nc.dram_tensor("W1", [D, F], F32, kind="ExternalInput")
    w2_d = nc.dram_tensor("W2", [F, D], F32, kind="ExternalInput")
    bkv_d = nc.dram_tensor("bkv_rep", [P, 2 * D], F32, kind="ExternalInput")
    bq_d = nc.dram_tensor("bq_rep", [P, D], F32, kind="ExternalInput")
    b1c_d = nc.dram_tensor("b1_cols", [P, FC], F32, kind="ExternalInput")
    alc_d = nc.dram_tensor("alpha_cols", [P, FC], F32, kind="ExternalInput")
    g_d = nc.dram_tensor("ln_g_rep", [P, D], F32, kind="ExternalInput")
    b_d = nc.dram_tensor("ln_b_rep", [P, D], F32, kind="ExternalInput")
    b2_d = nc.dram_tensor("b2_rep", [P, D], F32, kind="ExternalInput")
    iota_d = nc.dram_tensor("iota_rep", [P, CM * P], F16, kind="ExternalInput")
    ident_d = nc.dram_tensor("ident", [P, P], F32, kind="ExternalInput")
    eps_d = nc.dram_tensor("eps_col", [P, 1], F32, kind="ExternalInput")

    out_d = nc.dram_tensor("out", [NL, D], F32, kind="ExternalOutput")
    if debug:
        dbg_kv = nc.dram_tensor("dbg_kv", [NL, 2 * D], F16, kind="ExternalOutput")
        dbg_q = nc.dram_tensor("dbg_q", [NL, D], F16, kind="ExternalOutput")
        dbg_x1 = nc.dram_tensor("dbg_x1", [NL, D], F32, kind="ExternalOutput")
        dbg_kvg = nc.dram_tensor("dbg_kvg", [P, CM * 2 * D], F16, kind="ExternalOutput")
        dbg_ind = nc.dram_tensor("dbg_ind", [P, CM * P], F16, kind="ExternalOutput")
        dbg_mex = nc.dram_tensor("dbg_mex", [P, CM * (D + H)], F16, kind="ExternalOutput")
    kv_tab = nc.dram_tensor("kv_tab", [NP, 2 * D], F16)
    q_tab = nc.dram_tensor("q_tab", [NL, D], F16)

    scale = 1.0 / math.sqrt(D)

    with tile.TileContext(nc) as tc:
        with tc.tile_pool(name="consts", bufs=1) as cp:
            ident = cp.tile([P, P], F32, tag='ident')
            nc.sync.dma_start(ident[:], ident_d[:, :])
            ident16 = cp.tile([P, P], F16, tag='ident16')
            nc.vector.tensor_copy(ident16[:], ident[:])
            iota = cp.tile([P, CM * P], F16, tag='iota')
            nc.sync.dma_start(iota[:], iota_d[:, :])
            wq = cp.tile([P, D], F32, tag='wq')
            nc.sync.dma_start(wq[:], wq_d[:, :])
            wk = cp.tile([P, D], F32, tag='wk')
            nc.sync.dma_start(wk[:], wk_d[:, :])
            wv = cp.tile([P, D], F32, tag='wv')
            nc.sync.dma_start(wv[:], wv_d[:, :])
            w1 = cp.tile([P, F], F32, tag='w1')
            nc.sync.dma_start(w1[:], w1_d[:, :])
            w2c = []
            for c in range(FC):
                w2c.append(cp.tile([P, D], F32, name=f"w2c{c}", tag=f"w2c{c}"))
                nc.sync.dma_start(w2c[c][:], w2_d[c * FC_P:(c + 1) * FC_P, :])
            bkv = cp.tile([P, 2 * D], F32, tag='bkv')
            nc.sync.dma_start(bkv[:], bkv_d[:, :])
            bq = cp.tile([P, D], F32, tag='bq')
            nc.sync.dma_start(bq[:], bq_d[:, :])
            b1c = cp.tile([P, FC], F32, tag='b1c')
            nc.sync.dma_start(b1c[:], b1c_d[:, :])
            alc = cp.tile([P, FC], F32, tag='alc')
            nc.sync.dma_start(alc[:], alc_d[:, :])
            g_rep = cp.tile([P, D], F32, tag='g_rep')
            nc.sync.dma_start(g_rep[:], g_d[:, :])
            b_rep = cp.tile([P, D], F32, tag='b_rep')
            nc.sync.dma_start(b_rep[:], b_d[:, :])
            b2_rep = cp.tile([P, D], F32, tag='b2_rep')
            nc.sync.dma_start(b2_rep[:], b2_d[:, :])
            eps_c = cp.tile([P, 1], F32, tag='eps_c')
            nc.sync.dma_start(eps_c[:], eps_d[:, :])
            ikv_all = cp.tile([P, T * CM], I32, tag='ikv')
            nc.sync.dma_start(ikv_all[:], idx_kv_d[:, :])
            iq_all = cp.tile([P, T * CM], I32, tag='iq')
            nc.sync.dma_start(iq_all[:], idx_q_d[:, :])
            drel_all = cp.tile([P, T * CM], F16, tag='drel')
            nc.sync.dma_start(drel_all[:], dst_rel_d[:, :])

            def qkv_chunk(ap_pool, aio_pool, feat_src, row0, dst_tab, w_list,
                          bias_tile, dup=False):
                """feat rows [row0:row0+P] -> (x@W + b) rows of dst_tab."""
                ft = aio_pool.tile([P, D], F32, tag="ft")
                nc.sync.dma_start(ft[:], feat_src[row0:row0 + P, :])
                pt = ap_pool.tile([P, P], F32, space="PSUM", tag="pt")
                nc.tensor.transpose(out=pt[:], in_=ft[:], identity=ident[:])
                fT = aio_pool.tile([P, P], F32, tag="fT")
                nc.scalar.copy(fT[:], pt[:])
                nmm = len(w_list)
                pk = ap_pool.tile([P, nmm * D], F32, space="PSUM", tag="pk")
                for i, w_t in enumerate(w_list):
                    nc.tensor.matmul(pk[:, i * D:(i + 1) * D], fT[:], w_t[:],
                                     start=True, stop=True)
                sb = aio_pool.tile([P, nmm * D], F16, tag="sb")
                nc.vector.tensor_add(sb[:], pk[:], bias_tile[:])
                nc.sync.dma_start(dst_tab[row0:row0 + P, :], sb[:])

            def body(it=None):
                # ---- phase A/B: build kv and q tables on device
                with tc.tile_pool(name="aio", bufs=4) as aio, \
                     tc.tile_pool(name="aps", bufs=4, space="PSUM") as aps:
                    for cch in range(NP // P):
                        qkv_chunk(aps, aio, feat_full, cch * P, kv_tab,
                                  [wk, wv], bkv)
                    for tch in range(T):
                        qkv_chunk(aps, aio, feat_loc, tch * P, q_tab,
                                  [wq], bq, dup=False)

                tc.strict_bb_all_engine_barrier()
                if debug:
                    nc.sync.dma_start(dbg_kv[:, :], kv_tab[0:NL, :])
                    nc.sync.dma_start(dbg_q[:, :], q_tab[:, :])

                # ---- phase C: per node-tile edge aggregation + FFN
                with tc.tile_pool(name="cg", bufs=2) as cg, \
                     tc.tile_pool(name="cm", bufs=2) as cmp_, \
                     tc.tile_pool(name="cn", bufs=3) as cn, \
                     tc.tile_pool(name="cps", bufs=2, space="PSUM") as cps, \
                     tc.tile_pool(name="cps2", bufs=2, space="PSUM") as cps2, \
                     tc.tile_pool(name="cps3", bufs=3, space="PSUM") as cps3:
                    for t in range(T):
                        ecol = slice(t * CM, (t + 1) * CM)
                        kv_g = cg.tile([P, CM * 2 * D], F16, tag="kv_g")
                        for j in range(CM):
                            nc.gpsimd.indirect_dma_start(
                                out=kv_g[:, j * 2 * D:(j + 1) * 2 * D],
                                out_offset=None, in_=kv_tab[:, :],
                                in_offset=IndirectOffsetOnAxis(
                                    ap=ikv_all[:, t * CM + j:t * CM + j + 1],
                                    axis=0))
                        q_tile = cg.tile([P, D], F16, tag="q_tile")
                        nc.sync.dma_start(q_tile[:], q_tab[t * P:(t + 1) * P, :])

                        # indicator: ind[e, j, n] = (dst_rel[e, j] == n)
                        ind = cmp_.tile([P, CM * P], F16, tag="ind")
                        nc.vector.tensor_tensor(
                            out=ind[:].rearrange("p (c n) -> p c n", n=P),
                            in0=drel_all[:, ecol].to_broadcast([P, CM, P]),
                            in1=iota[:].rearrange("p (c n) -> p c n", n=P),
                            op=mybir.AluOpType.is_equal)

                        # per-edge q via ind_T matmul; scores s = k * q_e
                        kv_v = kv_g[:].rearrange("p (c w) -> p c w", w=2 * D)
                        sp = cmp_.tile([P, CM * D], F16, tag="sp")
                        for j in range(CM):
                            tpi = cps3.tile([P, P], F16, space="PSUM",
                                            tag="scr", name="tpi")
                            nc.tensor.transpose(
                                out=tpi[:], in_=ind[:, j * P:(j + 1) * P],
                                identity=ident16[:])
                            indT = cmp_.tile([P, P], F16, tag="indT")
                            nc.scalar.copy(indT[:], tpi[:])
                            pqe = cps3.tile([P, D], F32, space="PSUM",
                                            tag="scr", name="pqe")
                            nc.tensor.matmul(pqe[:], indT[:], q_tile[:],
                                             start=True, stop=True)
                            nc.vector.tensor_tensor(
                                out=sp[:, j * D:(j + 1) * D],
                                in0=kv_v[:, j, 0:D], in1=pqe[:],
                                op=mybir.AluOpType.mult)
                        e8 = cmp_.tile([P, CM * H], F32, tag="e8")
                        nc.vector.tensor_reduce(
                            out=e8[:].rearrange("p (c h) -> p c h", h=H),
                            in_=sp[:].rearrange("p (c h d) -> p c h d",
                                                h=H, d=DH),
                            axis=mybir.AxisListType.X,
                            op=mybir.AluOpType.add)

                        # m_ex: [e, j, 0:128] = v * exp(e8*scale) bcast;
                        #       [e, j, 128:136] = exp(e8*scale)
                        mex = cmp_.tile([P, CM * (D + H)], F16, tag="mex")
                        mex_v = mex[:].rearrange("p (c w) -> p c w", w=D + H)
                        nc.scalar.activation(
                            out=mex_v[:, :, D:D + H],
                            in_=e8[:].rearrange("p (c h) -> p c h", h=H),
                            func=mybir.ActivationFunctionType.Exp,
                            scale=scale)
                        nc.vector.tensor_tensor(
                            out=mex_v[:, :, 0:D].rearrange(
                                "p c (h d) -> p c h d", d=DH),
                            in0=kv_v[:, :, D:2 * D].rearrange(
                                "p c (h d) -> p c h d", d=DH),
                            in1=mex_v[:, :, D:D + H].to_broadcast(
                                [P, CM, H, DH]),
                            op=mybir.AluOpType.mult)

                        # segment sums via indicator matmuls -> [n, D+H]
                        ud = cps.tile([P, D + H], F32, space="PSUM", tag="ud")
                        for j in range(CM):
                            nc.tensor.matmul(
                                ud[:], ind[:, j * P:(j + 1) * P],
                                mex[:, j * (D + H):(j + 1) * (D + H)],
                                start=(j == 0), stop=(j == CM - 1))

                        # rst = u / max(denom, tiny); x1 = rst + feat
                        dsb = cn.tile([P, H], F32, tag="dsb")
                        nc.vector.tensor_scalar_max(dsb[:], ud[:, D:D + H],
                                                    1e-30)
                        rd = cn.tile([P, H], F32, tag="rd")
                        nc.vector.reciprocal(rd[:], dsb[:])
                        ftl = cn.tile([P, D], F32, tag="ftl")
                        nc.sync.dma_start(ftl[:], feat_loc[t * P:(t + 1) * P, :])
                        x1 = cn.tile([P, D], F32, tag="x1")
                        nc.vector.tensor_tensor(
                            out=x1[:].rearrange("p (h d) -> p h d", d=DH),
                            in0=ud[:, 0:D].rearrange("p (h d) -> p h d", d=DH),
                            in1=rd[:].to_broadcast([P, H, DH]),
                            op=mybir.AluOpType.mult)
                        nc.vector.tensor_add(x1[:], x1[:], ftl[:])
                        if debug:
                            nc.sync.dma_start(dbg_x1[t * P:(t + 1) * P, :], x1[:])
                            if t == 0:
                                nc.sync.dma_start(dbg_kvg[:, :], kv_g[:])
                                nc.sync.dma_start(dbg_ind[:, :], ind[:])
                                nc.sync.dma_start(dbg_mex[:, :], mex[:])

                        def layernorm(x_in, tag):
                            st = cn.tile([P, 6], F32, tag=f"st{tag}")
                            nc.vector.bn_stats(out=st[:], in_=x_in[:])
                            mv = cn.tile([P, 2], F32, tag=f"mv{tag}")
                            nc.vector.bn_aggr(out=mv[:], in_=st[:])
                            sd = cn.tile([P, 2], F32, tag=f"sd{tag}")
                            nc.scalar.activation(
                                out=sd[:, 0:1], in_=mv[:, 1:2],
                                func=mybir.ActivationFunctionType.Sqrt,
                                bias=eps_c[:, 0:1])
                            nc.vector.reciprocal(sd[:, 1:2], sd[:, 0:1])
                            y_t = cn.tile([P, D], F32, tag=f"y{tag}")
                            nc.vector.tensor_scalar(
                                out=y_t[:], in0=x_in[:],
                                scalar1=mv[:, 0:1], scalar2=sd[:, 1:2],
                                op0=mybir.AluOpType.subtract,
                                op1=mybir.AluOpType.mult)
                            nc.vector.tensor_mul(y_t[:], y_t[:], g_rep[:])
                            nc.vector.tensor_add(y_t[:], y_t[:], b_rep[:])
                            return y_t

                        y = layernorm(x1, "a")

                        # FFN: h^T_c = prelu(W1_c^T @ y^T + b1_c)
                        pyt = cps2.tile([P, P], F32, space="PSUM", tag="np", name="pyt")
                        nc.tensor.transpose(out=pyt[:], in_=y[:],
                                            identity=ident[:])
                        yT = cn.tile([P, P], F32, tag="yT")
                        nc.scalar.copy(yT[:], pyt[:])
                        hts = []
                        for c in range(FC):
                            ph = cps2.tile([P, P], F32, space="PSUM", tag="np", name="ph")
                            nc.tensor.matmul(ph[:],
                                             w1[:, c * FC_P:(c + 1) * FC_P],
                                             yT[:], start=True, stop=True)
                            ht = cn.tile([P, P], F32, tag=f"ht{c}")
                            nc.scalar.activation(
                                out=ht[:], in_=ph[:],
                                func=mybir.ActivationFunctionType.Prelu,
                                bias=b1c[:, c:c + 1], alpha=alc[:, c:c + 1])
                            hts.append(ht)
                        pf = cps2.tile([P, D], F32, space="PSUM", tag="pf", bufs=1)
                        for c in range(FC):
                            nc.tensor.matmul(pf[:], hts[c][:], w2c[c][:],
                                             start=(c == 0), stop=(c == FC - 1))
                        x2 = cn.tile([P, D], F32, tag="x2")
                        nc.vector.tensor_add(x2[:], pf[:], y[:])
                        nc.vector.tensor_add(x2[:], x2[:], b2_rep[:])

                        o = layernorm(x2, "b")
                        nc.sync.dma_start(out_d[t * P:(t + 1) * P, :], o[:])

            if repeat == 1:
                body()
            else:
                with tc.For_i(0, repeat, 1) as it:
                    body(it)

    nc.compile()
    return nc


# ---------------------------------------------------------------- entry

_CACHE = {}


def _get_program(key, *args, **kw):
    if key not in _CACHE:
        _CACHE[key] = build_program(*args, **kw)
    return _CACHE[key]


def make_in_maps(inputs, meta):
    feat_pad = meta["feat_pad"]
    NL, T, CM, D = meta["NL"], meta["T"], meta["CM"], meta["D"]
    F = inputs["W1"].shape[1]
    FC = F // FC_P
    bkv_rep = _rep(np.concatenate([np.asarray(inputs["bk"], np.float32),
                                   np.asarray(inputs["bv"], np.float32)]))
    common = dict(
        Wq=np.asarray(inputs["Wq"], np.float32),
        Wk=np.asarray(inputs["Wk"], np.float32),
        Wv=np.asarray(inputs["Wv"], np.float32),
        W1=np.asarray(inputs["W1"], np.float32),
        W2=np.asarray(inputs["W2"], np.float32),
        bkv_rep=bkv_rep,
        bq_rep=_rep(inputs["bq"]),
        b1_cols=np.ascontiguousarray(
            np.asarray(inputs["b1"], np.float32).reshape(FC, FC_P).T),
        alpha_cols=np.ascontiguousarray(
            np.asarray(inputs["alpha"], np.float32).reshape(FC, FC_P).T),
        ln_g_rep=_rep(inputs["ln_g"]),
        ln_b_rep=_rep(inputs["ln_b"]),
        b2_rep=_rep(inputs["b2"]),
        iota_rep=np.ascontiguousarray(
            np.tile(np.arange(P, dtype=np.float16), (P, CM))),
        ident=np.eye(P, dtype=np.float32),
        eps_col=np.full((P, 1), 1e-5, np.float32),
        feat_full=feat_pad,
    )
    in_maps = []
    for c in range(NCORES):
        m = dict(common)
        m["feat_loc"] = np.ascontiguousarray(feat_pad[c * NL:(c + 1) * NL])
        m["idx_kv"] = meta["idx_kv"][c]
        m["idx_q"] = meta["idx_q"][c]
        m["dst_rel"] = meta["dst_rel"][c]
        in_maps.append(m)
    return in_maps


def kernel(**inputs):
    feat = np.asarray(inputs["feat"], np.float32)
    src = np.asarray(inputs["src"], np.int32)
    dst = np.asarray(inputs["dst"], np.int32)
    meta = _preprocess(feat, src, dst)
    N, D, NL, NP, T, CM = (meta["N"], meta["D"], meta["NL"], meta["NP"],
                           meta["T"], meta["CM"])
    F = np.asarray(inputs["W1"]).shape[1]
    nc = _get_program(("main", NP, NL, T, CM, D, F), NP, NL, T, CM, D, F)
    in_maps = make_in_maps(inputs, meta)
    res = run_bass_kernel_spmd(nc, in_maps, core_ids=list(range(NCORES)))
    out = np.concatenate([res.results[c]["out"] for c in range(NCORES)], axis=0)
    return out[:N].astype(np.float32)
